# revision 13
# baseline (speedup 1.0000x reference)
"""CAAN (cross-asset attention) Trainium2 kernel, v6.

Reference computation (B=32, N=2048, D=256):
    q = x@Wq + bq;  k = x@Wk + bk;  v = x@Wv + bv
    beta = softmax(q @ k^T / sqrt(D), axis=-1)
    out  = (beta @ v) @ Ww + bw            # [B, N]

Algebra (host): effective logits l_ij/16 + d_j with l_ij = y_i . x_j,
    y = x @ (Wq Wk^T), d = s * x.(Wk bq), and
    out_i = sum_j E_ij u_j / sum_j E_ij + (bv.Ww + bw),  u = x @ (Wv Ww).

Device per core (4 batches), [j, i] layout, j-tile-parity hybrid:
  scores even jt: fp16 matmul pair (128-contraction each, accumulating)
          -> st0 [128j, 512i] PSUM; 216 ns/MM sustained.
  scores odd jt:  ONE fp8e4 DoubleRow matmul (256-contraction in 512
          cycles) -> st1; 216 ns. fp8 quantization noise lands only on
          half the j's, keeping the softmax ratio error ~1.6e-2.
  exp even jt (ScalarE): native Exp, FD=1024 over both i-blocks, bf16
          out, d_j applied via per-partition bias AP (free).
  exp odd jt (DVE): 2^x bit-hack uint16(l*a+b) bitcast bf16, constant
          scalars (d_j folded into the reduce weights instead - a
          per-partition scalar AP costs +150 ns/op on DVE).
  reduce: 4 col-tiled bf16 matmuls per jp (tile_position=(0,32c)), all
          four (i2, parity) chains sharing ONE psum bank in 16-row
          slices; rhs = E tile, lhsT = (u0, 1) [even jt] or
          (u0 e^d, e^d) [odd jt] rows; accumulated over the 8 jps of an
          h unit; 216 ns per concurrent quad. The four jp==0 start=True
          MMs are interleaved between the next jp's score MMs (full-array
          col-group conflict serializes them) because concurrent
          start=True MMs sharing a bank race on the per-partition
          has_written clears. Per h unit one FD-512 copy (alternating
          ScalarE/DVE) moves the bank to SBUF; tiny DMAs stream out.
Host: out = nv/dv + const. Data-parallel over B (4 batches per core).
PSUM: 2x [128,2,512] ACT score tiles + 3x [128,512] DVE score tiles +
1 reduce bank = 8 banks.
"""

import ml_dtypes
import numpy as np

import concourse.bass as bass
import concourse.bacc as bacc
import concourse.tile as tile
from concourse import mybir
from concourse.bass_utils import run_bass_kernel_spmd

B, N, D = 32, 2048, 256
NCORES = 8
BPC = B // NCORES
P = 128
FB = 512
NH = 2           # ib-pair units per batch
NJP = 8          # jt-pair units per h
NO = N // 2      # odd/even half width

F32 = mybir.dt.float32
FP16 = mybir.dt.float16
BF16 = mybir.dt.bfloat16
F8 = mybir.dt.float8e4
U16 = mybir.dt.uint16
DRM = mybir.MatmulPerfMode.DoubleRow
E4M3 = ml_dtypes.float8_e4m3
BFML = ml_dtypes.bfloat16

LOG2E = float(np.log2(np.e))
A16 = 8.0 * LOG2E                       # = 128 * log2e / 16
B16 = 127.0 * 128.0 - 0.043 * 128.0     # schraudolph offset in bf16 bits

_CACHE = {}
LAST_EXEC_NS = None


def _build_program():
    nc = bacc.Bacc("TRN2")

    x8d = nc.dram_tensor("x8o", [BPC, P, 2, NO], F8, kind="ExternalInput")
    x16d = nc.dram_tensor("x16e", [BPC, P, 2, NO], FP16, kind="ExternalInput")
    y8d = nc.dram_tensor("y8", [BPC, P, 2, N], F8, kind="ExternalInput")
    y16d = nc.dram_tensor("y16", [BPC, P, 2, N], FP16, kind="ExternalInput")
    uod = nc.dram_tensor("uo", [P, BPC, 16, 16], BF16, kind="ExternalInput")
    dad = nc.dram_tensor("da", [P, BPC, 16], F32, kind="ExternalInput")
    sr = nc.dram_tensor("sr", [BPC, NH, 2, 2, 2, FB], F32, kind="ExternalOutput")

    with tile.TileContext(nc) as tc:
        with (
            tc.tile_pool(name="consts", bufs=1) as consts,
            tc.tile_pool(name="xp", bufs=2) as xp,
            tc.tile_pool(name="xp16", bufs=2) as xp16,
            tc.tile_pool(name="yp", bufs=2) as yp,
            tc.tile_pool(name="yp16", bufs=2) as yp16,
            tc.tile_pool(name="ppp", bufs=4) as ppp,
            tc.tile_pool(name="rcp", bufs=2) as rcp,
            tc.tile_pool(name="ps0", bufs=2, space="PSUM") as ps0,
            tc.tile_pool(name="ps1", bufs=3, space="PSUM") as ps1,
            tc.tile_pool(name="psr", bufs=1, space="PSUM") as psr,
        ):
            uo_sb = consts.tile([P, BPC, 16, 16], BF16)
            da_sb = consts.tile([P, BPC, 16], F32)
            nc.sync.dma_start(out=uo_sb, in_=uod[:, :, :, :])
            nc.sync.dma_start(out=da_sb, in_=dad[:, :, :])

            red = psr.tile([P, FB], F32, tag="red", name="red")
            z8 = consts.tile([P, 1], F8)
            nc.vector.memset(z8, 0)

            def chain_mm(b, h, jp, pp, c):
                i2, sj = c // 2, c % 2
                jt = 2 * jp + sj
                nc.tensor.matmul(
                    red[32 * c:32 * c + 16, :],
                    lhsT=uo_sb[:, b, jt, :],
                    rhs=pp[sj][:, i2, :],
                    start=False, stop=(jp == NJP - 1),
                    tile_position=(0, 32 * c),
                    skip_group_check=True,
                )

            def clear_red(xo):
                # full-partition start=True MM multiplying by a zero vector:
                # clears every partition's has_written bits for the bank and
                # writes 0.0 into column 0 (chains accumulate onto it).
                nc.tensor.matmul(red[:, 0:1], lhsT=xo[:, 0, 0:P], rhs=z8,
                                 start=True, stop=True, skip_group_check=True)

            def emit_copy(b, h):
                eng = (b * NH + h) % 2
                rc = rcp.tile([P, FB], F32, tag="rc", name=f"rc_{b}_{h}")
                if eng == 0:
                    nc.scalar.copy(out=rc, in_=red)
                else:
                    nc.vector.tensor_copy(out=rc, in_=red)
                for i2 in range(2):
                    for sj in range(2):
                        c = 2 * i2 + sj
                        nc.sync.dma_start(
                            out=sr[b, h, i2, sj, :, :],
                            in_=rc[32 * c:32 * c + 2, :])

            pend = []  # (b, h, jp, pp)

            xt_last = None
            for b in range(BPC):
                xo = xp.tile([P, 2, NO], F8, name=f"xo_{b}")
                xe = xp16.tile([P, 2, NO], FP16)
                yt8 = yp.tile([P, 2, N], F8)
                yt16 = yp16.tile([P, 2, N], FP16)
                if b == 0:
                    nc.scalar.dma_start(out=xo[:, :, 0:NO // 4], in_=x8d[b, :, :, 0:NO // 4])
                    nc.scalar.dma_start(out=xe[:, :, 0:NO // 4], in_=x16d[b, :, :, 0:NO // 4])
                    nc.sync.dma_start(out=yt16[:, :, 0:FB], in_=y16d[b, :, :, 0:FB])
                    nc.scalar.dma_start(out=yt16[:, :, FB:2 * FB], in_=y16d[b, :, :, FB:2 * FB])
                    nc.gpsimd.dma_start(out=yt8[:, :, 0:2 * FB], in_=y8d[b, :, :, 0:2 * FB])
                    nc.sync.dma_start(out=yt16[:, :, 2 * FB:N], in_=y16d[b, :, :, 2 * FB:N])
                    nc.gpsimd.dma_start(out=yt8[:, :, 2 * FB:N], in_=y8d[b, :, :, 2 * FB:N])
                    nc.scalar.dma_start(out=xe[:, :, NO // 4:NO], in_=x16d[b, :, :, NO // 4:NO])
                    nc.scalar.dma_start(out=xo[:, :, NO // 4:NO], in_=x8d[b, :, :, NO // 4:NO])
                else:
                    nc.sync.dma_start(out=yt16[:, :, 0:N // 2], in_=y16d[b, :, :, 0:N // 2])
                    nc.sync.dma_start(out=yt16[:, :, N // 2:N], in_=y16d[b, :, :, N // 2:N])
                    nc.scalar.dma_start(out=xe[:, :, 0:NO // 2], in_=x16d[b, :, :, 0:NO // 2])
                    nc.scalar.dma_start(out=xe[:, :, NO // 2:NO], in_=x16d[b, :, :, NO // 2:NO])
                    nc.gpsimd.dma_start(out=xo[:, :, 0:NO // 2], in_=x8d[b, :, :, 0:NO // 2])
                    nc.gpsimd.dma_start(out=xo[:, :, NO // 2:NO], in_=x8d[b, :, :, NO // 2:NO])
                    # derive fp8 y from the fp16 copy on the idle gpsimd engine
                    nc.gpsimd.tensor_copy(out=yt8[:, :, 0:N // 2],
                                          in_=yt16[:, :, 0:N // 2])
                    nc.gpsimd.tensor_copy(out=yt8[:, :, N // 2:N],
                                          in_=yt16[:, :, N // 2:N])

                for h in range(NH):
                    for jp in range(NJP):
                        # issue the lag-2 pending quad BEFORE this jp's scores
                        # so the exp->quad dependency chain stays off the
                        # critical path (exp(jp) must not sem-wait on quads).
                        ent = pend.pop(0) if len(pend) >= 2 else None
                        if ent is not None:
                            if ent[2] == 0:
                                clear_red(xo if ent[0] == b else xt_last)
                            for c in range(4):
                                chain_mm(ent[0], ent[1], ent[2], ent[3], c)
                            if ent[2] == NJP - 1:
                                emit_copy(ent[0], ent[1])

                        st0 = ps0.tile([P, 2, FB], F32, tag="st0",
                                       name=f"st0_{b}_{h}_{jp}")
                        st1 = [ps1.tile([P, FB], F32, tag="st1",
                                        name=f"st1_{b}_{h}_{jp}_{i2}")
                               for i2 in range(2)]
                        # odd-jt fp8 DR MMs first (frees DVE rotation)
                        for i2 in range(2):
                            ib = 2 * h + i2
                            nc.tensor.matmul(
                                st1[i2],
                                lhsT=xo[:, :, jp * P:(jp + 1) * P],
                                rhs=yt8[:, :, ib * FB:(ib + 1) * FB],
                                start=True, stop=True,
                                perf_mode=DRM,
                            )
                        # even-jt fp16 MM pairs
                        for i2 in range(2):
                            ib = 2 * h + i2
                            for k in range(2):
                                nc.tensor.matmul(
                                    st0[:, i2, :],
                                    lhsT=xe[:, k, jp * P:(jp + 1) * P],
                                    rhs=yt16[:, k, ib * FB:(ib + 1) * FB],
                                    start=(k == 0), stop=(k == 1),
                                )

                        # separate ACT/DVE pp tiles: a shared tile would make
                        # tile serialize the two engines' writes every jp.
                        ppA = ppp.tile([P, 2, FB], BF16, tag="ppA",
                                       name=f"ppA_{b}_{h}_{jp}")
                        ppD = ppp.tile([P, 2, FB], BF16, tag="ppD",
                                       name=f"ppD_{b}_{h}_{jp}")
                        nc.vector.tensor_scalar(
                            out=ppD[:, 0, :].bitcast(U16), in0=st1[0],
                            scalar1=A16, scalar2=B16,
                            op0=mybir.AluOpType.mult,
                            op1=mybir.AluOpType.add,
                        )
                        nc.vector.tensor_scalar(
                            out=ppD[:, 1, :].bitcast(U16), in0=st1[1],
                            scalar1=A16, scalar2=B16,
                            op0=mybir.AluOpType.mult,
                            op1=mybir.AluOpType.add,
                        )
                        jt0 = 2 * jp
                        nc.scalar.activation(
                            out=ppA, in_=st0,
                            func=mybir.ActivationFunctionType.Exp,
                            bias=da_sb[:, b, jt0:jt0 + 1], scale=0.0625,
                        )
                        pend.append((b, h, jp, (ppA, ppD)))
                xt_last = xo

            while pend:
                ent = pend.pop(0)
                if ent[2] == 0:
                    clear_red(xt_last)
                for c in range(4):
                    chain_mm(ent[0], ent[1], ent[2], ent[3], c)
                if ent[2] == NJP - 1:
                    emit_copy(ent[0], ent[1])

    nc.compile()
    return nc


def kernel(x, Wq, bq, Wk, bk, Wv, bv, Ww, bw, trace=False):
    global LAST_EXEC_NS
    x = np.asarray(x, dtype=np.float32)
    Wq = np.asarray(Wq, dtype=np.float32)
    bq = np.asarray(bq, dtype=np.float32)
    Wk = np.asarray(Wk, dtype=np.float32)
    bk = np.asarray(bk, dtype=np.float32)
    Wv = np.asarray(Wv, dtype=np.float32)
    bv = np.asarray(bv, dtype=np.float32)
    Ww = np.asarray(Ww, dtype=np.float32)
    bw = np.asarray(bw, dtype=np.float32)

    s = np.float32(1.0 / np.sqrt(D))
    A = (Wq @ Wk.T) * (16.0 * s)
    xf = x.reshape(B * N, D)
    yt = (xf @ A).reshape(B, N, D)
    u0 = (xf @ (Wv @ Ww))[:, 0].reshape(B, N)
    d = ((xf @ (Wk @ bq)) * s).reshape(B, N)
    const_add = float(bv @ Ww[:, 0]) + float(bw[0])

    # [b, p, k, n] with contraction index = k*128 + p
    def tr(a):
        return np.ascontiguousarray(a.reshape(B, -1, 2, P).transpose(0, 3, 2, 1))

    xb = x.reshape(B, 16, P, D)
    x8o_all = tr(xb[:, 1::2].reshape(B, NO, D)).astype(E4M3)
    x16e_all = tr(xb[:, 0::2].reshape(B, NO, D)).astype(np.float16)
    y8_all = tr(yt).astype(E4M3)
    y16_all = tr(yt).astype(np.float16)

    # uo [P, B, 16jt, 16]: odd jt fold e^d into weights; even jt (u0, 1)
    ed = np.exp(d.astype(np.float64)).astype(np.float32)
    u0T = u0.reshape(B, 16, P).transpose(2, 0, 1)       # [P, B, 16]
    edT = ed.reshape(B, 16, P).transpose(2, 0, 1)
    uo_all = np.zeros((P, B, 16, 16), dtype=BFML)
    uo_all[:, :, 0::2, 0] = u0T[:, :, 0::2]
    uo_all[:, :, 0::2, 1] = 1.0
    uo_all[:, :, 1::2, 0] = (u0T * edT)[:, :, 1::2]
    uo_all[:, :, 1::2, 1] = edT[:, :, 1::2]
    da_all = np.ascontiguousarray(
        d.reshape(B, 16, P).transpose(2, 0, 1).astype(np.float32))

    if "nc" not in _CACHE:
        _CACHE["nc"] = _build_program()
    nc = _CACHE["nc"]

    in_maps = []
    for c in range(NCORES):
        sl = slice(c * BPC, (c + 1) * BPC)
        in_maps.append({
            "x8o": np.ascontiguousarray(x8o_all[sl]),
            "x16e": np.ascontiguousarray(x16e_all[sl]),
            "y8": np.ascontiguousarray(y8_all[sl]),
            "y16": np.ascontiguousarray(y16_all[sl]),
            "uo": np.ascontiguousarray(uo_all[:, sl]),
            "da": np.ascontiguousarray(da_all[:, sl]),
        })

    res = run_bass_kernel_spmd(nc, in_maps, core_ids=list(range(NCORES)), trace=trace)
    LAST_EXEC_NS = res.exec_time_ns

    out = np.empty((B, N), dtype=np.float32)
    for c in range(NCORES):
        srv = res.results[c]["sr"].astype(np.float64)  # [BPC, NH, 2, 2, 2, FB]
        for bb in range(BPC):
            for h in range(NH):
                for i2 in range(2):
                    ib = 2 * h + i2
                    nv = srv[bb, h, i2, 0, 0] + srv[bb, h, i2, 1, 0]
                    dv = srv[bb, h, i2, 0, 1] + srv[bb, h, i2, 1, 1]
                    out[c * BPC + bb, ib * FB:(ib + 1) * FB] = \
                        (nv / dv + const_add).astype(np.float32)
    return out


# revision 15
# speedup vs baseline: 1.0385x; 1.0385x over previous
"""CAAN (cross-asset attention) Trainium2 kernel, v6.

Reference computation (B=32, N=2048, D=256):
    q = x@Wq + bq;  k = x@Wk + bk;  v = x@Wv + bv
    beta = softmax(q @ k^T / sqrt(D), axis=-1)
    out  = (beta @ v) @ Ww + bw            # [B, N]

Algebra (host): effective logits l_ij/16 + d_j with l_ij = y_i . x_j,
    y = x @ (Wq Wk^T), d = s * x.(Wk bq), and
    out_i = sum_j E_ij u_j / sum_j E_ij + (bv.Ww + bw),  u = x @ (Wv Ww).

Device per core (4 batches), [j, i] layout, j-tile-parity hybrid:
  scores even jt: fp16 matmul pair (128-contraction each, accumulating)
          -> st0 [128j, 512i] PSUM; 216 ns/MM sustained.
  scores odd jt:  ONE fp8e4 DoubleRow matmul (256-contraction in 512
          cycles) -> st1; 216 ns. fp8 quantization noise lands only on
          half the j's, keeping the softmax ratio error ~1.6e-2.
  exp even jt (ScalarE): native Exp, FD=1024 over both i-blocks, bf16
          out, d_j applied via per-partition bias AP (free).
  exp odd jt (DVE): 2^x bit-hack uint16(l*a+b) bitcast bf16, constant
          scalars (d_j folded into the reduce weights instead - a
          per-partition scalar AP costs +150 ns/op on DVE).
  reduce: 4 col-tiled bf16 matmuls per jp (tile_position=(0,32c)), all
          four (i2, parity) chains sharing ONE psum bank in 16-row
          slices; rhs = E tile, lhsT = (u0, 1) [even jt] or
          (u0 e^d, e^d) [odd jt] rows; accumulated over the 8 jps of an
          h unit; 216 ns per concurrent quad. The four jp==0 start=True
          MMs are interleaved between the next jp's score MMs (full-array
          col-group conflict serializes them) because concurrent
          start=True MMs sharing a bank race on the per-partition
          has_written clears. Per h unit one FD-512 copy (alternating
          ScalarE/DVE) moves the bank to SBUF; tiny DMAs stream out.
Host: out = nv/dv + const. Data-parallel over B (4 batches per core).
PSUM: 2x [128,2,512] ACT score tiles + 3x [128,512] DVE score tiles +
1 reduce bank = 8 banks.
"""

import ml_dtypes
import numpy as np

import concourse.bass as bass
import concourse.bacc as bacc
import concourse.tile as tile
from concourse import mybir
from concourse.bass_utils import run_bass_kernel_spmd

B, N, D = 32, 2048, 256
NCORES = 8
BPC = B // NCORES
P = 128
FB = 512
NH = 2           # ib-pair units per batch
NJP = 8          # jt-pair units per h
NO = N // 2      # odd/even half width

F32 = mybir.dt.float32
FP16 = mybir.dt.float16
BF16 = mybir.dt.bfloat16
F8 = mybir.dt.float8e4
U16 = mybir.dt.uint16
DRM = mybir.MatmulPerfMode.DoubleRow
E4M3 = ml_dtypes.float8_e4m3
BFML = ml_dtypes.bfloat16

LOG2E = float(np.log2(np.e))
A16 = 8.0 * LOG2E                       # = 128 * log2e / 16
B16 = 127.0 * 128.0 - 0.043 * 128.0     # schraudolph offset in bf16 bits

_CACHE = {}
LAST_EXEC_NS = None


def _build_program():
    nc = bacc.Bacc("TRN2")

    x8d = nc.dram_tensor("x8o", [BPC, P, 2, NO], F8, kind="ExternalInput")
    x16d = nc.dram_tensor("x16e", [BPC, P, 2, NO], FP16, kind="ExternalInput")
    y8d = nc.dram_tensor("y8", [BPC, P, 2, N], F8, kind="ExternalInput")
    y16d = nc.dram_tensor("y16", [BPC, P, 2, N], FP16, kind="ExternalInput")
    uod = nc.dram_tensor("uo", [P, BPC, 16, 16], BF16, kind="ExternalInput")
    dad = nc.dram_tensor("da", [P, BPC, 16], F32, kind="ExternalInput")
    sr = nc.dram_tensor("sr", [BPC, NH, 2, 2, 2, FB], F32, kind="ExternalOutput")

    with tile.TileContext(nc) as tc:
        with (
            tc.tile_pool(name="consts", bufs=1) as consts,
            tc.tile_pool(name="xp", bufs=2) as xp,
            tc.tile_pool(name="xp16", bufs=2) as xp16,
            tc.tile_pool(name="yp", bufs=2) as yp,
            tc.tile_pool(name="yp16", bufs=2) as yp16,
            tc.tile_pool(name="ppp", bufs=4) as ppp,
            tc.tile_pool(name="rcp", bufs=2) as rcp,
            tc.tile_pool(name="ps0", bufs=2, space="PSUM") as ps0,
            tc.tile_pool(name="ps1", bufs=3, space="PSUM") as ps1,
            tc.tile_pool(name="psr", bufs=1, space="PSUM") as psr,
        ):
            uo_sb = consts.tile([P, BPC, 16, 16], BF16)
            da_sb = consts.tile([P, BPC, 16], F32)
            nc.sync.dma_start(out=uo_sb, in_=uod[:, :, :, :])
            nc.sync.dma_start(out=da_sb, in_=dad[:, :, :])

            red = psr.tile([P, FB], F32, tag="red", name="red")
            z8 = consts.tile([P, 1], F8)
            nc.vector.memset(z8, 0)

            def chain_mm(b, h, jp, pp, c):
                i2, sj = c // 2, c % 2
                jt = 2 * jp + sj
                nc.tensor.matmul(
                    red[32 * c:32 * c + 16, :],
                    lhsT=uo_sb[:, b, jt, :],
                    rhs=pp[sj][:, i2, :],
                    start=False, stop=(jp == NJP - 1),
                    tile_position=(0, 32 * c),
                    skip_group_check=True,
                )

            def clear_red(xo):
                # full-partition start=True MM multiplying by a zero vector:
                # clears every partition's has_written bits for the bank and
                # writes 0.0 into column 0 (chains accumulate onto it).
                nc.tensor.matmul(red[:, 0:1], lhsT=xo[:, 0, 0:P], rhs=z8,
                                 start=True, stop=True, skip_group_check=True)

            def emit_copy(b, h):
                eng = (b * NH + h) % 2
                rc = rcp.tile([P, FB], F32, tag="rc", name=f"rc_{b}_{h}")
                if eng == 0:
                    nc.scalar.copy(out=rc, in_=red)
                else:
                    nc.vector.tensor_copy(out=rc, in_=red)
                for i2 in range(2):
                    for sj in range(2):
                        c = 2 * i2 + sj
                        nc.sync.dma_start(
                            out=sr[b, h, i2, sj, :, :],
                            in_=rc[32 * c:32 * c + 2, :])

            pend = []  # (b, h, jp, pp)

            # HAM warm-up: ~24 garbage matmuls into the (to-be-cleared) red
            # bank while batch-0 DMAs are in flight, so the PE clock is at
            # 8/8 when real compute starts. uo_sb is tiny and arrives first.
            warm = consts.tile([P, FB], BF16, name="warm")
            nc.vector.memset(warm, 0)
            for i in range(24):
                nc.tensor.matmul(red[0:16, :], lhsT=uo_sb[:, 0, i % 16, :],
                                 rhs=warm, start=True, stop=True,
                                 skip_group_check=True)

            xt_last = None
            for b in range(BPC):
                xo = xp.tile([P, 2, NO], F8, name=f"xo_{b}")
                xe = xp16.tile([P, 2, NO], FP16)
                yt8 = yp.tile([P, 2, N], F8)
                yt16 = yp16.tile([P, 2, N], FP16)
                if b == 0:
                    nc.sync.dma_start(out=yt16[:, :, 0:2 * FB], in_=y16d[b, :, :, 0:2 * FB])
                    nc.scalar.dma_start(out=xo[:, :, 0:NO // 4], in_=x8d[b, :, :, 0:NO // 4])
                    nc.scalar.dma_start(out=xe[:, :, 0:NO // 4], in_=x16d[b, :, :, 0:NO // 4])
                    nc.gpsimd.dma_start(out=yt8[:, :, 0:2 * FB], in_=y8d[b, :, :, 0:2 * FB])
                    nc.scalar.dma_start(out=xe[:, :, NO // 4:NO], in_=x16d[b, :, :, NO // 4:NO])
                    nc.scalar.dma_start(out=xo[:, :, NO // 4:NO], in_=x8d[b, :, :, NO // 4:NO])
                    nc.sync.dma_start(out=yt16[:, :, 2 * FB:N], in_=y16d[b, :, :, 2 * FB:N])
                    nc.gpsimd.dma_start(out=yt8[:, :, 2 * FB:N], in_=y8d[b, :, :, 2 * FB:N])
                else:
                    nc.sync.dma_start(out=yt16[:, :, 0:N // 2], in_=y16d[b, :, :, 0:N // 2])
                    nc.sync.dma_start(out=yt16[:, :, N // 2:N], in_=y16d[b, :, :, N // 2:N])
                    nc.scalar.dma_start(out=xe[:, :, 0:NO // 2], in_=x16d[b, :, :, 0:NO // 2])
                    nc.scalar.dma_start(out=xe[:, :, NO // 2:NO], in_=x16d[b, :, :, NO // 2:NO])
                    nc.gpsimd.dma_start(out=xo[:, :, 0:NO // 2], in_=x8d[b, :, :, 0:NO // 2])
                    nc.gpsimd.dma_start(out=xo[:, :, NO // 2:NO], in_=x8d[b, :, :, NO // 2:NO])
                    # derive fp8 y from the fp16 copy on the idle gpsimd engine
                    nc.gpsimd.tensor_copy(out=yt8[:, :, 0:N // 2],
                                          in_=yt16[:, :, 0:N // 2])
                    nc.gpsimd.tensor_copy(out=yt8[:, :, N // 2:N],
                                          in_=yt16[:, :, N // 2:N])

                for h in range(NH):
                    for jp in range(NJP):
                        # issue the lag-2 pending quad BEFORE this jp's scores
                        # so the exp->quad dependency chain stays off the
                        # critical path (exp(jp) must not sem-wait on quads).
                        ent = pend.pop(0) if len(pend) >= 2 else None
                        if ent is not None:
                            if ent[2] == 0:
                                clear_red(xo if ent[0] == b else xt_last)
                            for c in range(4):
                                chain_mm(ent[0], ent[1], ent[2], ent[3], c)
                            if ent[2] == NJP - 1:
                                emit_copy(ent[0], ent[1])

                        st0 = ps0.tile([P, 2, FB], F32, tag="st0",
                                       name=f"st0_{b}_{h}_{jp}")
                        st1 = [ps1.tile([P, FB], F32, tag="st1",
                                        name=f"st1_{b}_{h}_{jp}_{i2}")
                               for i2 in range(2)]
                        # DR1, fp16 x4, DR2: each DR LDWEIGHTS (213 ns)
                        # hides behind preceding matmuls
                        def dr_mm(i2):
                            ib = 2 * h + i2
                            nc.tensor.matmul(
                                st1[i2],
                                lhsT=xo[:, :, jp * P:(jp + 1) * P],
                                rhs=yt8[:, :, ib * FB:(ib + 1) * FB],
                                start=True, stop=True,
                                perf_mode=DRM,
                            )
                        dr_mm(0)
                        for i2 in range(2):
                            ib = 2 * h + i2
                            for k in range(2):
                                nc.tensor.matmul(
                                    st0[:, i2, :],
                                    lhsT=xe[:, k, jp * P:(jp + 1) * P],
                                    rhs=yt16[:, k, ib * FB:(ib + 1) * FB],
                                    start=(k == 0), stop=(k == 1),
                                )
                        dr_mm(1)

                        # separate ACT/DVE pp tiles: a shared tile would make
                        # tile serialize the two engines' writes every jp.
                        ppA = ppp.tile([P, 2, FB], BF16, tag="ppA",
                                       name=f"ppA_{b}_{h}_{jp}")
                        ppD = ppp.tile([P, 2, FB], BF16, tag="ppD",
                                       name=f"ppD_{b}_{h}_{jp}")
                        nc.vector.tensor_scalar(
                            out=ppD[:, 0, :].bitcast(U16), in0=st1[0],
                            scalar1=A16, scalar2=B16,
                            op0=mybir.AluOpType.mult,
                            op1=mybir.AluOpType.add,
                        )
                        nc.vector.tensor_scalar(
                            out=ppD[:, 1, :].bitcast(U16), in0=st1[1],
                            scalar1=A16, scalar2=B16,
                            op0=mybir.AluOpType.mult,
                            op1=mybir.AluOpType.add,
                        )
                        jt0 = 2 * jp
                        nc.scalar.activation(
                            out=ppA, in_=st0,
                            func=mybir.ActivationFunctionType.Exp,
                            bias=da_sb[:, b, jt0:jt0 + 1], scale=0.0625,
                        )
                        pend.append((b, h, jp, (ppA, ppD)))
                xt_last = xo

            while pend:
                ent = pend.pop(0)
                if ent[2] == 0:
                    clear_red(xt_last)
                for c in range(4):
                    chain_mm(ent[0], ent[1], ent[2], ent[3], c)
                if ent[2] == NJP - 1:
                    emit_copy(ent[0], ent[1])

    nc.compile()
    return nc


def kernel(x, Wq, bq, Wk, bk, Wv, bv, Ww, bw, trace=False):
    global LAST_EXEC_NS
    x = np.asarray(x, dtype=np.float32)
    Wq = np.asarray(Wq, dtype=np.float32)
    bq = np.asarray(bq, dtype=np.float32)
    Wk = np.asarray(Wk, dtype=np.float32)
    bk = np.asarray(bk, dtype=np.float32)
    Wv = np.asarray(Wv, dtype=np.float32)
    bv = np.asarray(bv, dtype=np.float32)
    Ww = np.asarray(Ww, dtype=np.float32)
    bw = np.asarray(bw, dtype=np.float32)

    s = np.float32(1.0 / np.sqrt(D))
    A = (Wq @ Wk.T) * (16.0 * s)
    xf = x.reshape(B * N, D)
    yt = (xf @ A).reshape(B, N, D)
    u0 = (xf @ (Wv @ Ww))[:, 0].reshape(B, N)
    d = ((xf @ (Wk @ bq)) * s).reshape(B, N)
    const_add = float(bv @ Ww[:, 0]) + float(bw[0])

    # [b, p, k, n] with contraction index = k*128 + p
    def tr(a):
        return np.ascontiguousarray(a.reshape(B, -1, 2, P).transpose(0, 3, 2, 1))

    xb = x.reshape(B, 16, P, D)
    x8o_all = tr(xb[:, 1::2].reshape(B, NO, D)).astype(E4M3)
    x16e_all = tr(xb[:, 0::2].reshape(B, NO, D)).astype(np.float16)
    y8_all = tr(yt).astype(E4M3)
    y16_all = tr(yt).astype(np.float16)

    # uo [P, B, 16jt, 16]: odd jt fold e^d into weights; even jt (u0, 1)
    ed = np.exp(d.astype(np.float64)).astype(np.float32)
    u0T = u0.reshape(B, 16, P).transpose(2, 0, 1)       # [P, B, 16]
    edT = ed.reshape(B, 16, P).transpose(2, 0, 1)
    uo_all = np.zeros((P, B, 16, 16), dtype=BFML)
    uo_all[:, :, 0::2, 0] = u0T[:, :, 0::2]
    uo_all[:, :, 0::2, 1] = 1.0
    uo_all[:, :, 1::2, 0] = (u0T * edT)[:, :, 1::2]
    uo_all[:, :, 1::2, 1] = edT[:, :, 1::2]
    da_all = np.ascontiguousarray(
        d.reshape(B, 16, P).transpose(2, 0, 1).astype(np.float32))

    if "nc" not in _CACHE:
        _CACHE["nc"] = _build_program()
    nc = _CACHE["nc"]

    in_maps = []
    for c in range(NCORES):
        sl = slice(c * BPC, (c + 1) * BPC)
        in_maps.append({
            "x8o": np.ascontiguousarray(x8o_all[sl]),
            "x16e": np.ascontiguousarray(x16e_all[sl]),
            "y8": np.ascontiguousarray(y8_all[sl]),
            "y16": np.ascontiguousarray(y16_all[sl]),
            "uo": np.ascontiguousarray(uo_all[:, sl]),
            "da": np.ascontiguousarray(da_all[:, sl]),
        })

    res = run_bass_kernel_spmd(nc, in_maps, core_ids=list(range(NCORES)), trace=trace)
    LAST_EXEC_NS = res.exec_time_ns

    out = np.empty((B, N), dtype=np.float32)
    for c in range(NCORES):
        srv = res.results[c]["sr"].astype(np.float64)  # [BPC, NH, 2, 2, 2, FB]
        for bb in range(BPC):
            for h in range(NH):
                for i2 in range(2):
                    ib = 2 * h + i2
                    nv = srv[bb, h, i2, 0, 0] + srv[bb, h, i2, 1, 0]
                    dv = srv[bb, h, i2, 0, 1] + srv[bb, h, i2, 1, 1]
                    out[c * BPC + bb, ib * FB:(ib + 1) * FB] = \
                        (nv / dv + const_add).astype(np.float32)
    return out
